# revision 4
# baseline (speedup 1.0000x reference)
"""ARMA GNN kernel for 8 trn2 NeuronCores (self-contained).

Math (validated vs reference in numpy, rel err ~2e-6):
  A = D^-1/2 Adj D^-1/2 over target nodes; S(y)[d] = sum_{e->d} y[src_e]
  layer1 (T=2, relu): T1R1 = [x|1] @ W1a ; P = dinv*S(dinv*T)
     out0 = relu(P1 + R1) ; T2 = out0 @ blockdiag(w1_w) ; out1 = relu(P2 + R1)
  layer2+pool+head collapse (all linear): pqr = out1 @ M
     s1 = S(dinv*p) ; y = dinv*(dinv*s1 + q + dbar) ; s2 = S(y)
     node_scalar = dinv*s2 + r + ebar ; out[g] = sum_{n in g} node_scalar + bg

Distribution: nodes/edges sharded by destination node across 8 cores,
weights replicated, per-node tables all-gathered, propagation via
dma_gather (1024-idx chunks) + one-hot matmul segment reduction.

SPMD uniformity: each core packs its 12800 node slots into 160 blocks of 80
real slots such that each block receives <=256 edges from each of the 4
source-table chunks; every (pass, block) segment is padded to exactly 256
slots so the instruction stream is identical on every core.
"""
import numpy as np

import concourse.bass as bass
import concourse.bacc as bacc
import concourse.mybir as mybir
import concourse.tile as tile
from concourse.bass_utils import run_bass_kernel_spmd
from concourse.masks import make_identity

N, E, G = 100000, 1200000, 2048
FIN, H, FOUT, K = 75, 16, 64, 3
NC = 8
SH = N // NC            # 12500 real nodes per core
CNT = 80                # node slots per block (table rows per block)
NB = 160                # blocks per core
NLOC = NB * CNT         # 12800 node slots per core
SEG = 256               # slots per (pass, block) segment
NSC = 4                 # source table chunks (2 core-shards each)
CH = 1024               # idxs per dma_gather instruction
CHUNKS_PER_PASS = NB * SEG // CH   # 40
ROWS_SHARD = NLOC                  # 12800 table rows per core shard
ROWS_CHUNK = 2 * ROWS_SHARD        # 25600 rows per source chunk (< 32768)
S_TOT = NSC * NB * SEG             # 163840 slots per round
KH = K * H
F32 = mybir.dt.float32
I16 = mybir.dt.int16
OP = mybir.AluOpType

_graph_cache = {}
TRACE = False            # test harness can enable NTFF timing
LAST_EXEC_NS = None


def _pack_blocks(deg_vec):
    """Assign SH real nodes to (block, rel): CNT slots/block, per-chunk edge
    load <= SEG.  deg_vec [SH, NSC]."""
    order = np.argsort(-deg_vec.sum(axis=1), kind="stable")
    loads = np.zeros((NB, NSC), np.int64)
    counts = np.zeros(NB, np.int64)
    blk = np.empty(SH, np.int64)
    rel = np.empty(SH, np.int64)
    open_list = list(range(NB))
    for n in order:
        d = deg_vec[n]
        best, bestscore = -1, None
        for b in open_list:
            nl = loads[b] + d
            mx = nl.max()
            if mx > SEG:
                continue
            if bestscore is None or mx < bestscore:
                best, bestscore = b, mx
                if mx <= SEG // 2:
                    break
        assert best >= 0, "block packing failed; lower CNT"
        b = best
        blk[n] = b
        rel[n] = counts[b]
        counts[b] += 1
        loads[b] += d
        if counts[b] >= CNT:
            open_list.remove(b)
    return blk, rel


def _host_prep(x, edge_index, batch, w):
    row = edge_index[0].astype(np.int64)
    col = edge_index[1].astype(np.int64)
    batch = batch.astype(np.int64)
    deg = np.bincount(col, minlength=N).astype(np.float32)
    dinv = np.where(deg > 0, deg ** -0.5, 0.0).astype(np.float32)

    w1i, w1w, w1r, w1b = w["w1_init"], w["w1_w"], w["w1_root"], w["w1_bias"]
    w2i, w2w, w2r, w2b = w["w2_init"], w["w2_w"], w["w2_root"], w["w2_bias"]
    wg, bg = w["wg"], w["bg"]
    w1a = np.zeros((FIN + 1, 2 * KH), np.float32)
    w1wbd = np.zeros((KH, KH), np.float32)
    for k in range(K):
        w1a[:FIN, k * H:(k + 1) * H] = w1i[k]
        w1a[:FIN, KH + k * H:KH + (k + 1) * H] = w1r[k]
        w1a[FIN, KH + k * H:KH + (k + 1) * H] = w1b[k, 0]
        w1wbd[k * H:(k + 1) * H, k * H:(k + 1) * H] = w1w[k]
    abar = np.mean([w2i[k] @ w2w[k] @ wg for k in range(K)], axis=0)
    bbar = np.mean([w2r[k] @ w2w[k] @ wg for k in range(K)], axis=0)
    gbar = np.mean([w2r[k] @ wg for k in range(K)], axis=0)
    dbar = float(np.mean([(w2b[k] @ w2w[k] @ wg).item() for k in range(K)]))
    ebar = float(np.mean([(w2b[k] @ wg).item() for k in range(K)]))
    pqrM = np.zeros((KH, 3), np.float32)
    for k in range(K):
        pqrM[k * H:(k + 1) * H, 0] = abar[:, 0] / K
        pqrM[k * H:(k + 1) * H, 1] = bbar[:, 0] / K
        pqrM[k * H:(k + 1) * H, 2] = gbar[:, 0] / K

    xa = np.concatenate([x.astype(np.float32), np.ones((N, 1), np.float32)],
                        axis=1)

    # pack blocks per core; build global node -> table row map
    g_rowloc = np.empty(N, np.int64)
    packs = []
    for c in range(NC):
        lo = c * SH
        m = (col >= lo) & (col < lo + SH)
        src_c, dst_c = row[m], col[m] - lo
        sc_c = src_c // (2 * SH)
        deg_vec = np.zeros((SH, NSC), np.int64)
        np.add.at(deg_vec, (dst_c, sc_c), 1)
        blk, rel = _pack_blocks(deg_vec)
        g_rowloc[lo:lo + SH] = blk * CNT + rel
        packs.append((src_c, dst_c, sc_c, blk, rel))

    # one dummy (all-zero) row per core shard for pad slots
    pad_row = np.zeros(NC, np.int64)
    for c in range(NC):
        used = np.zeros(NLOC, bool)
        used[g_rowloc[c * SH:(c + 1) * SH]] = True
        pad_row[c] = int(np.flatnonzero(~used)[0])

    cores = []
    for c in range(NC):
        src_c, dst_c, sc_c, blk, rel = packs[c]
        dblk, drel = blk[dst_c], rel[dst_c]
        idx_arr = np.zeros(S_TOT, np.int64)
        rel_arr = np.full(S_TOT, -5.0, np.float32)
        for p in range(NSC):
            mm = sc_c == p
            s_src, s_dblk, s_drel = src_c[mm], dblk[mm], drel[mm]
            o = np.argsort(s_dblk, kind="stable")
            s_src, s_dblk, s_drel = s_src[o], s_dblk[o], s_drel[o]
            cnts = np.bincount(s_dblk, minlength=NB)
            assert cnts.max() <= SEG, f"core {c} pass {p}: {cnts.max()}"
            starts = np.zeros(NB, np.int64)
            starts[1:] = np.cumsum(cnts)[:-1]
            base = p * NB * SEG
            slots = base + s_dblk * SEG + (np.arange(len(s_dblk)) - starts[s_dblk])
            src_core = s_src // SH
            idx_arr[slots] = (src_core % 2) * ROWS_SHARD + g_rowloc[s_src]
            rel_arr[slots] = s_drel
            padmask = np.ones(NB * SEG, bool)
            padmask[slots - base] = False
            idx_arr[base + np.flatnonzero(padmask)] = pad_row[2 * p]
        iw = np.zeros((32, S_TOT // 16), np.int16)
        ar = np.arange(S_TOT)
        iw[ar % 16, ar // 16] = idx_arr.astype(np.int16)
        iw[16 + ar % 16, ar // 16] = idx_arr.astype(np.int16)
        relm = np.zeros((128, S_TOT // 128), np.float32)
        relm[ar % 128, ar // 128] = rel_arr

        nid_blk = np.full((128, NB), -1, np.int64)
        nid_blk[rel, blk] = c * SH + np.arange(SH)
        real = nid_blk >= 0
        safe = np.clip(nid_blk, 0, N - 1)
        dinv_blk = np.where(real, dinv[safe], 0.0).astype(np.float32)

        xbT = np.zeros((FIN + 1, NB * 128), np.float32)
        xbT[:, (blk * 128 + rel)] = xa[c * SH:(c + 1) * SH].T

        cores.append(dict(idx=iw, rel=relm, dinv=dinv_blk, xbT=xbT,
                          nid=nid_blk, real=real))

    shared = dict(w1a=w1a, w1wbd=w1wbd, pqrM=pqrM, dbar=dbar, ebar=ebar,
                  bg=float(np.asarray(bg).ravel()[0]))
    return cores, shared


def _spmm_round(nc, psA, gpool, tbl, idx_sb, rel_sb, iota8_sb, accum, width):
    nc.vector.memset(accum[:], 0.0)
    for p in range(NSC):
        tblc = tbl[p * ROWS_CHUNK:(p + 1) * ROWS_CHUNK, :]
        for cch in range(CHUNKS_PER_PASS):
            ci = p * CHUNKS_PER_PASS + cch
            gath = gpool.tile([128, 8 * 64], F32, tag="gath")
            nc.gpsimd.dma_gather(
                out_ap=gath[:].rearrange("p (g d) -> p g d", d=64),
                in_ap=tblc,
                idxs_ap=idx_sb[:, ci * (CH // 16):(ci + 1) * (CH // 16)],
                num_idxs=CH, num_idxs_reg=CH, elem_size=64,
                prepare_only=False,
            )
            oh = gpool.tile([128, 8 * 128], F32, tag="oh")
            nc.vector.tensor_tensor(
                out=oh[:].rearrange("p (g m) -> p g m", m=128),
                in0=iota8_sb[:].rearrange("p (g m) -> p g m", m=128),
                in1=rel_sb[:, ci * 8:(ci + 1) * 8]
                    .rearrange("p (g o) -> p g o", o=1)
                    .to_broadcast([128, 8, 128]),
                op=OP.is_equal,
            )
            for half in range(4):
                blk_id = cch * 4 + half
                ps = psA.tile([128, 64], F32, tag="segps")
                for sub in range(2):
                    g = half * 2 + sub
                    nc.tensor.matmul(
                        out=ps[:, 0:width],
                        lhsT=oh[:, g * 128:(g + 1) * 128],
                        rhs=gath[:, g * 64:g * 64 + width],
                        start=(sub == 0), stop=(sub == 1),
                    )
                acc = accum[:, blk_id * width:(blk_id + 1) * width]
                nc.vector.tensor_tensor(out=acc, in0=acc, in1=ps[:, 0:width],
                                        op=OP.add)


def _build_graph(dbar, ebar):
    nc = bacc.Bacc("TRN2", target_bir_lowering=False, debug=False,
                   num_devices=NC)
    idx_in = nc.dram_tensor("idx", [32, S_TOT // 16], I16, kind="ExternalInput")
    rel_in = nc.dram_tensor("rel", [128, S_TOT // 128], F32, kind="ExternalInput")
    dinv_in = nc.dram_tensor("dinv", [128, NB], F32, kind="ExternalInput")
    xbT_in = nc.dram_tensor("xbT", [FIN + 1, NB * 128], F32, kind="ExternalInput")
    w1a_in = nc.dram_tensor("w1a", [FIN + 1, 2 * KH], F32, kind="ExternalInput")
    w1wbd_in = nc.dram_tensor("w1wbd", [KH, KH], F32, kind="ExternalInput")
    pqrM_in = nc.dram_tensor("pqrM", [KH, 3], F32, kind="ExternalInput")
    iota8_in = nc.dram_tensor("iota8", [128, 8 * 128], F32, kind="ExternalInput")
    out_ext = nc.dram_tensor("out", [128, NB], F32, kind="ExternalOutput")
    tshard = nc.dram_tensor("tshard_w", [ROWS_SHARD, 64], F32)
    tbl = nc.dram_tensor("tbl", [NC * ROWS_SHARD, 64], F32, addr_space="Shared")

    with tile.TileContext(nc) as tc:
        with tc.tile_pool(name="const", bufs=1) as cpool, \
             tc.tile_pool(name="big", bufs=1) as bigp, \
             tc.tile_pool(name="work", bufs=3) as gpool, \
             tc.tile_pool(name="psA", bufs=2, space="PSUM") as psA, \
             tc.tile_pool(name="psB", bufs=1, space="PSUM") as psB:
            idx_sb = cpool.tile([32, S_TOT // 16], I16)
            rel_sb = cpool.tile([128, S_TOT // 128], F32)
            dinv_sb = cpool.tile([128, NB], F32)
            w1a_sb = cpool.tile([FIN + 1, 2 * KH], F32)
            w1wbd_sb = cpool.tile([KH, KH], F32)
            pqrM_sb = cpool.tile([KH, 3], F32)
            iota8_sb = cpool.tile([128, 8 * 128], F32)
            ident_sb = cpool.tile([128, 128], F32)
            for dst, src in ((idx_sb, idx_in), (rel_sb, rel_in),
                             (dinv_sb, dinv_in), (w1a_sb, w1a_in),
                             (w1wbd_sb, w1wbd_in), (pqrM_sb, pqrM_in),
                             (iota8_sb, iota8_in)):
                nc.sync.dma_start(out=dst[:], in_=src[:])
            make_identity(nc, ident_sb[:])

            accum = bigp.tile([128, NB * 48], F32)
            R1 = bigp.tile([128, NB * 48], F32)
            out01 = bigp.tile([128, NB * 48], F32)
            saccum = bigp.tile([128, NB], F32)
            ns_sb = bigp.tile([128, NB], F32)
            pqr_sb = bigp.tile([128, NB * 3], F32)

            # phase A: T1R1; table <- dinv*T1; keep R1
            for b in range(NB):
                xbt = gpool.tile([FIN + 1, 128], F32, tag="xbt")
                nc.sync.dma_start(out=xbt[:], in_=xbT_in[:, b * 128:(b + 1) * 128])
                ps = psA.tile([128, 2 * KH], F32, tag="mm")
                nc.tensor.matmul(out=ps[:], lhsT=xbt[:], rhs=w1a_sb[:],
                                 start=True, stop=True)
                ev = gpool.tile([128, 48], F32, tag="ev")
                nc.vector.tensor_scalar_mul(out=ev[:], in0=ps[:, 0:KH],
                                            scalar1=dinv_sb[:, b:b + 1])
                nc.sync.dma_start(out=tshard[b * CNT:(b + 1) * CNT, 0:KH],
                                  in_=ev[0:CNT, :])
                nc.vector.tensor_copy(out=R1[:, b * 48:(b + 1) * 48],
                                      in_=ps[:, KH:2 * KH])

            def allgather():
                nc.gpsimd.collective_compute(
                    "AllGather", OP.bypass, replica_groups=[list(range(NC))],
                    ins=[tshard[:]], outs=[tbl[:]])

            def post_prop(dst):
                a3 = accum[:].rearrange("p (b f) -> p b f", f=48)
                d3 = (dinv_sb[:].rearrange("p (b o) -> p b o", o=1)
                      .to_broadcast([128, NB, 48]))
                nc.vector.tensor_tensor(out=a3, in0=a3, in1=d3, op=OP.mult)
                nc.vector.tensor_tensor(out=dst[:], in0=accum[:], in1=R1[:],
                                        op=OP.add)
                nc.vector.tensor_scalar_max(out=dst[:], in0=dst[:], scalar1=0.0)

            allgather()
            _spmm_round(nc, psA, gpool, tbl, idx_sb, rel_sb, iota8_sb, accum, 48)
            post_prop(out01)

            # T2 = out0 @ w1wbd -> table
            for b in range(NB):
                pst = psB.tile([KH, 128], F32, tag="tr")
                nc.tensor.transpose(out=pst[:], in_=out01[:, b * 48:(b + 1) * 48],
                                    identity=ident_sb[:])
                sbt = gpool.tile([KH, 128], F32, tag="sbt")
                nc.vector.tensor_copy(out=sbt[:], in_=pst[:])
                ps2 = psB.tile([128, KH], F32, tag="mm2")
                nc.tensor.matmul(out=ps2[:], lhsT=sbt[:], rhs=w1wbd_sb[:],
                                 start=True, stop=True)
                ev = gpool.tile([128, 48], F32, tag="ev")
                nc.vector.tensor_scalar_mul(out=ev[:], in0=ps2[:],
                                            scalar1=dinv_sb[:, b:b + 1])
                nc.sync.dma_start(out=tshard[b * CNT:(b + 1) * CNT, 0:KH],
                                  in_=ev[0:CNT, :])

            allgather()
            _spmm_round(nc, psA, gpool, tbl, idx_sb, rel_sb, iota8_sb, accum, 48)
            post_prop(out01)

            # pqr = out1 @ pqrM
            for b in range(NB):
                pst = psB.tile([KH, 128], F32, tag="tr")
                nc.tensor.transpose(out=pst[:], in_=out01[:, b * 48:(b + 1) * 48],
                                    identity=ident_sb[:])
                sbt = gpool.tile([KH, 128], F32, tag="sbt")
                nc.vector.tensor_copy(out=sbt[:], in_=pst[:])
                ps3 = psB.tile([128, 3], F32, tag="mm3")
                nc.tensor.matmul(out=ps3[:], lhsT=sbt[:], rhs=pqrM_sb[:],
                                 start=True, stop=True)
                nc.vector.tensor_copy(out=pqr_sb[:, b * 3:(b + 1) * 3],
                                      in_=ps3[:])

            pqr3 = pqr_sb[:].rearrange("p (b f) -> p b f", f=3)

            def write_scalar_table(src_tile):
                nc.sync.dma_start(
                    out=tshard[:].rearrange("(b r) d -> r b d", r=CNT)[:, :, 0:1],
                    in_=src_tile[0:CNT, :].rearrange("r (b o) -> r b o", o=1))

            # spmv 1: table col0 = dinv * p
            pval = gpool.tile([128, NB], F32, tag="pv")
            nc.vector.tensor_tensor(out=pval[:], in0=pqr3[:, :, 0],
                                    in1=dinv_sb[:], op=OP.mult)
            write_scalar_table(pval)
            allgather()
            _spmm_round(nc, psA, gpool, tbl, idx_sb, rel_sb, iota8_sb, saccum, 1)

            # y = dinv*(dinv*s1 + q + dbar)
            yv = gpool.tile([128, NB], F32, tag="pv")
            nc.vector.tensor_tensor(out=yv[:], in0=saccum[:], in1=dinv_sb[:],
                                    op=OP.mult)
            qd = gpool.tile([128, NB], F32, tag="qd")
            nc.vector.tensor_scalar(out=qd[:], in0=pqr3[:, :, 1], scalar1=dbar,
                                    scalar2=None, op0=OP.add)
            nc.vector.tensor_tensor(out=yv[:], in0=yv[:], in1=qd[:], op=OP.add)
            nc.vector.tensor_tensor(out=yv[:], in0=yv[:], in1=dinv_sb[:],
                                    op=OP.mult)
            write_scalar_table(yv)
            allgather()
            _spmm_round(nc, psA, gpool, tbl, idx_sb, rel_sb, iota8_sb, saccum, 1)

            # node_scalar = dinv*s2 + r + ebar
            nc.vector.tensor_tensor(out=ns_sb[:], in0=saccum[:], in1=dinv_sb[:],
                                    op=OP.mult)
            rv = gpool.tile([128, NB], F32, tag="qd")
            nc.vector.tensor_scalar(out=rv[:], in0=pqr3[:, :, 2], scalar1=ebar,
                                    scalar2=None, op0=OP.add)
            nc.vector.tensor_tensor(out=ns_sb[:], in0=ns_sb[:], in1=rv[:],
                                    op=OP.add)
            nc.sync.dma_start(out=out_ext[:], in_=ns_sb[:])

    nc.compile()
    return nc


def kernel(**inputs):
    x = np.asarray(inputs["x"], np.float32)
    edge_index = np.asarray(inputs["edge_index"])
    batch = np.asarray(inputs["batch"]).astype(np.int64)
    w = {kk: np.asarray(vv, np.float32) for kk, vv in inputs.items()
         if kk not in ("x", "edge_index", "batch")}
    cores, shared = _host_prep(x, edge_index, batch, w)

    if "nc" not in _graph_cache:
        _graph_cache["nc"] = _build_graph(shared["dbar"], shared["ebar"])
    nc = _graph_cache["nc"]

    iota8 = np.broadcast_to(
        np.tile(np.arange(128, dtype=np.float32), 8)[None, :],
        (128, 8 * 128)).copy()
    in_maps = []
    for c in range(NC):
        d = cores[c]
        in_maps.append({
            "idx": d["idx"], "rel": d["rel"], "dinv": d["dinv"],
            "xbT": d["xbT"], "w1a": shared["w1a"], "w1wbd": shared["w1wbd"],
            "pqrM": shared["pqrM"], "iota8": iota8,
        })
    global LAST_EXEC_NS
    res = run_bass_kernel_spmd(nc, in_maps, core_ids=list(range(NC)),
                               trace=TRACE)
    LAST_EXEC_NS = res.exec_time_ns

    pooled = np.zeros(G, np.float64)
    for c in range(NC):
        ns = res.results[c]["out"]           # [128, NB]
        real = cores[c]["real"]
        nid = cores[c]["nid"]
        gids = batch[nid[real]]
        pooled += np.bincount(gids, weights=ns[real].astype(np.float64),
                              minlength=G)
    out = pooled.astype(np.float32)[:, None] + shared["bg"]
    return out.astype(np.float32)


# revision 5
# speedup vs baseline: 1.0108x; 1.0108x over previous
"""ARMA GNN kernel for 8 trn2 NeuronCores (self-contained).

Math (validated vs reference in numpy, rel err ~2e-6):
  A = D^-1/2 Adj D^-1/2 over target nodes; S(y)[d] = sum_{e->d} y[src_e]
  layer1 (T=2, relu): T1R1 = [x|1] @ W1a ; P = dinv*S(dinv*T)
     out0 = relu(P1 + R1) ; T2 = out0 @ blockdiag(w1_w) ; out1 = relu(P2 + R1)
  layer2+pool+head collapse (all linear): pqr = out1 @ M
     s1 = S(dinv*p) ; y = dinv*(dinv*s1 + q + dbar) ; s2 = S(y)
     node_scalar = dinv*s2 + r + ebar ; out[g] = sum_{n in g} node_scalar + bg

Distribution: nodes/edges sharded by destination node across 8 cores,
weights replicated, per-node tables all-gathered, propagation via
dma_gather (1024-idx chunks) + one-hot matmul segment reduction.

SPMD uniformity: each core packs its 12800 node slots into 160 blocks of 80
real slots such that each block receives <=256 edges from each of the 4
source-table chunks; every (pass, block) segment is padded to exactly 256
slots so the instruction stream is identical on every core.
"""
import numpy as np

import concourse.bass as bass
import concourse.bacc as bacc
import concourse.mybir as mybir
import concourse.tile as tile
from concourse.bass_utils import run_bass_kernel_spmd
from concourse.masks import make_identity

N, E, G = 100000, 1200000, 2048
FIN, H, FOUT, K = 75, 16, 64, 3
NC = 8
SH = N // NC            # 12500 real nodes per core
CNT = 80                # node slots per block (table rows per block)
NB = 160                # blocks per core
NLOC = NB * CNT         # 12800 node slots per core
SEG = 256               # slots per (pass, block) segment
NSC = 4                 # source table chunks (2 core-shards each)
CH = 1024               # idxs per dma_gather instruction
CHUNKS_PER_PASS = NB * SEG // CH   # 40
ROWS_SHARD = NLOC                  # 12800 table rows per core shard
ROWS_CHUNK = 2 * ROWS_SHARD        # 25600 rows per source chunk (< 32768)
S_TOT = NSC * NB * SEG             # 163840 slots per round
KH = K * H
F32 = mybir.dt.float32
BF16 = mybir.dt.bfloat16
I16 = mybir.dt.int16
OP = mybir.AluOpType

_graph_cache = {}
TRACE = False            # test harness can enable NTFF timing
LAST_EXEC_NS = None


def _pack_blocks(deg_vec):
    """Assign SH real nodes to (block, rel): CNT slots/block, per-chunk edge
    load <= SEG.  deg_vec [SH, NSC]."""
    order = np.argsort(-deg_vec.sum(axis=1), kind="stable")
    loads = np.zeros((NB, NSC), np.int64)
    counts = np.zeros(NB, np.int64)
    blk = np.empty(SH, np.int64)
    rel = np.empty(SH, np.int64)
    open_list = list(range(NB))
    for n in order:
        d = deg_vec[n]
        best, bestscore = -1, None
        for b in open_list:
            nl = loads[b] + d
            mx = nl.max()
            if mx > SEG:
                continue
            if bestscore is None or mx < bestscore:
                best, bestscore = b, mx
                if mx <= SEG // 2:
                    break
        assert best >= 0, "block packing failed; lower CNT"
        b = best
        blk[n] = b
        rel[n] = counts[b]
        counts[b] += 1
        loads[b] += d
        if counts[b] >= CNT:
            open_list.remove(b)
    return blk, rel


def _host_prep(x, edge_index, batch, w):
    row = edge_index[0].astype(np.int64)
    col = edge_index[1].astype(np.int64)
    batch = batch.astype(np.int64)
    deg = np.bincount(col, minlength=N).astype(np.float32)
    dinv = np.where(deg > 0, deg ** -0.5, 0.0).astype(np.float32)

    w1i, w1w, w1r, w1b = w["w1_init"], w["w1_w"], w["w1_root"], w["w1_bias"]
    w2i, w2w, w2r, w2b = w["w2_init"], w["w2_w"], w["w2_root"], w["w2_bias"]
    wg, bg = w["wg"], w["bg"]
    w1a = np.zeros((FIN + 1, 2 * KH), np.float32)
    w1wbd = np.zeros((KH, KH), np.float32)
    for k in range(K):
        w1a[:FIN, k * H:(k + 1) * H] = w1i[k]
        w1a[:FIN, KH + k * H:KH + (k + 1) * H] = w1r[k]
        w1a[FIN, KH + k * H:KH + (k + 1) * H] = w1b[k, 0]
        w1wbd[k * H:(k + 1) * H, k * H:(k + 1) * H] = w1w[k]
    abar = np.mean([w2i[k] @ w2w[k] @ wg for k in range(K)], axis=0)
    bbar = np.mean([w2r[k] @ w2w[k] @ wg for k in range(K)], axis=0)
    gbar = np.mean([w2r[k] @ wg for k in range(K)], axis=0)
    dbar = float(np.mean([(w2b[k] @ w2w[k] @ wg).item() for k in range(K)]))
    ebar = float(np.mean([(w2b[k] @ wg).item() for k in range(K)]))
    pqrM = np.zeros((KH, 3), np.float32)
    for k in range(K):
        pqrM[k * H:(k + 1) * H, 0] = abar[:, 0] / K
        pqrM[k * H:(k + 1) * H, 1] = bbar[:, 0] / K
        pqrM[k * H:(k + 1) * H, 2] = gbar[:, 0] / K

    xa = np.concatenate([x.astype(np.float32), np.ones((N, 1), np.float32)],
                        axis=1)

    # pack blocks per core; build global node -> table row map
    g_rowloc = np.empty(N, np.int64)
    packs = []
    for c in range(NC):
        lo = c * SH
        m = (col >= lo) & (col < lo + SH)
        src_c, dst_c = row[m], col[m] - lo
        sc_c = src_c // (2 * SH)
        deg_vec = np.zeros((SH, NSC), np.int64)
        np.add.at(deg_vec, (dst_c, sc_c), 1)
        blk, rel = _pack_blocks(deg_vec)
        g_rowloc[lo:lo + SH] = blk * CNT + rel
        packs.append((src_c, dst_c, sc_c, blk, rel))

    # one dummy (all-zero) row per core shard for pad slots
    pad_row = np.zeros(NC, np.int64)
    for c in range(NC):
        used = np.zeros(NLOC, bool)
        used[g_rowloc[c * SH:(c + 1) * SH]] = True
        pad_row[c] = int(np.flatnonzero(~used)[0])

    cores = []
    for c in range(NC):
        src_c, dst_c, sc_c, blk, rel = packs[c]
        dblk, drel = blk[dst_c], rel[dst_c]
        idx_arr = np.zeros(S_TOT, np.int64)
        rel_arr = np.full(S_TOT, -5.0, np.float32)
        for p in range(NSC):
            mm = sc_c == p
            s_src, s_dblk, s_drel = src_c[mm], dblk[mm], drel[mm]
            o = np.argsort(s_dblk, kind="stable")
            s_src, s_dblk, s_drel = s_src[o], s_dblk[o], s_drel[o]
            cnts = np.bincount(s_dblk, minlength=NB)
            assert cnts.max() <= SEG, f"core {c} pass {p}: {cnts.max()}"
            starts = np.zeros(NB, np.int64)
            starts[1:] = np.cumsum(cnts)[:-1]
            base = p * NB * SEG
            slots = base + s_dblk * SEG + (np.arange(len(s_dblk)) - starts[s_dblk])
            src_core = s_src // SH
            idx_arr[slots] = (src_core % 2) * ROWS_SHARD + g_rowloc[s_src]
            rel_arr[slots] = s_drel
            padmask = np.ones(NB * SEG, bool)
            padmask[slots - base] = False
            idx_arr[base + np.flatnonzero(padmask)] = pad_row[2 * p]
        iw = np.zeros((32, S_TOT // 16), np.int16)
        ar = np.arange(S_TOT)
        iw[ar % 16, ar // 16] = idx_arr.astype(np.int16)
        iw[16 + ar % 16, ar // 16] = idx_arr.astype(np.int16)
        import ml_dtypes
        relm = np.zeros((128, S_TOT // 128), ml_dtypes.bfloat16)
        relm[ar % 128, ar // 128] = rel_arr.astype(ml_dtypes.bfloat16)

        nid_blk = np.full((128, NB), -1, np.int64)
        nid_blk[rel, blk] = c * SH + np.arange(SH)
        real = nid_blk >= 0
        safe = np.clip(nid_blk, 0, N - 1)
        dinv_blk = np.where(real, dinv[safe], 0.0).astype(np.float32)

        xbT = np.zeros((FIN + 1, NB * 128), np.float32)
        xbT[:, (blk * 128 + rel)] = xa[c * SH:(c + 1) * SH].T

        cores.append(dict(idx=iw, rel=relm, dinv=dinv_blk, xbT=xbT,
                          nid=nid_blk, real=real))

    shared = dict(w1a=w1a, w1wbd=w1wbd, pqrM=pqrM, dbar=dbar, ebar=ebar,
                  bg=float(np.asarray(bg).ravel()[0]))
    return cores, shared


def _spmm_round(nc, psA, gpool, tbl, idx_sb, rel_sb, iota8_sb, accum, width):
    nc.vector.memset(accum[:], 0.0)
    for p in range(NSC):
        tblc = tbl[p * ROWS_CHUNK:(p + 1) * ROWS_CHUNK, :]
        for cch in range(CHUNKS_PER_PASS):
            ci = p * CHUNKS_PER_PASS + cch
            gath = gpool.tile([128, 8 * 128], BF16, tag="gath")
            nc.gpsimd.dma_gather(
                out_ap=gath[:].rearrange("p (g d) -> p g d", d=128),
                in_ap=tblc,
                idxs_ap=idx_sb[:, ci * (CH // 16):(ci + 1) * (CH // 16)],
                num_idxs=CH, num_idxs_reg=CH, elem_size=128,
                prepare_only=False,
            )
            oh = gpool.tile([128, 8 * 128], BF16, tag="oh")
            nc.vector.tensor_tensor(
                out=oh[:].rearrange("p (g m) -> p g m", m=128),
                in0=iota8_sb[:].rearrange("p (g m) -> p g m", m=128),
                in1=rel_sb[:, ci * 8:(ci + 1) * 8]
                    .rearrange("p (g o) -> p g o", o=1)
                    .to_broadcast([128, 8, 128]),
                op=OP.is_equal,
            )
            for half in range(4):
                blk_id = cch * 4 + half
                ps = psA.tile([128, 64], F32, tag="segps")
                for sub in range(2):
                    g = half * 2 + sub
                    nc.tensor.matmul(
                        out=ps[:, 0:width],
                        lhsT=oh[:, g * 128:(g + 1) * 128],
                        rhs=gath[:, g * 128:g * 128 + width],
                        start=(sub == 0), stop=(sub == 1),
                    )
                if width == 2:
                    acc = saccum_view = accum[:, blk_id:blk_id + 1]
                    nc.vector.tensor_tensor(out=acc, in0=acc, in1=ps[:, 0:1],
                                            op=OP.add)
                    nc.vector.tensor_tensor(out=acc, in0=acc, in1=ps[:, 1:2],
                                            op=OP.add)
                else:
                    acc = accum[:, blk_id * width:(blk_id + 1) * width]
                    nc.vector.tensor_tensor(out=acc, in0=acc,
                                            in1=ps[:, 0:width], op=OP.add)


def _build_graph(dbar, ebar):
    nc = bacc.Bacc("TRN2", target_bir_lowering=False, debug=False,
                   num_devices=NC)
    idx_in = nc.dram_tensor("idx", [32, S_TOT // 16], I16, kind="ExternalInput")
    rel_in = nc.dram_tensor("rel", [128, S_TOT // 128], BF16, kind="ExternalInput")
    dinv_in = nc.dram_tensor("dinv", [128, NB], F32, kind="ExternalInput")
    xbT_in = nc.dram_tensor("xbT", [FIN + 1, NB * 128], F32, kind="ExternalInput")
    w1a_in = nc.dram_tensor("w1a", [FIN + 1, 2 * KH], F32, kind="ExternalInput")
    w1wbd_in = nc.dram_tensor("w1wbd", [KH, KH], F32, kind="ExternalInput")
    pqrM_in = nc.dram_tensor("pqrM", [KH, 3], F32, kind="ExternalInput")
    iota8_in = nc.dram_tensor("iota8", [128, 8 * 128], BF16, kind="ExternalInput")
    out_ext = nc.dram_tensor("out", [128, NB], F32, kind="ExternalOutput")
    tshard = nc.dram_tensor("tshard_w", [ROWS_SHARD, 128], BF16)
    tbl = nc.dram_tensor("tbl", [NC * ROWS_SHARD, 128], BF16, addr_space="Shared")

    with tile.TileContext(nc) as tc:
        with tc.tile_pool(name="const", bufs=1) as cpool, \
             tc.tile_pool(name="big", bufs=1) as bigp, \
             tc.tile_pool(name="work", bufs=3) as gpool, \
             tc.tile_pool(name="psA", bufs=2, space="PSUM") as psA, \
             tc.tile_pool(name="psB", bufs=1, space="PSUM") as psB:
            idx_sb = cpool.tile([32, S_TOT // 16], I16)
            rel_sb = cpool.tile([128, S_TOT // 128], BF16)
            dinv_sb = cpool.tile([128, NB], F32)
            w1a_sb = cpool.tile([FIN + 1, 2 * KH], F32)
            w1wbd_sb = cpool.tile([KH, KH], F32)
            pqrM_sb = cpool.tile([KH, 3], F32)
            iota8_sb = cpool.tile([128, 8 * 128], BF16)
            ident_sb = cpool.tile([128, 128], F32)
            for dst, src in ((idx_sb, idx_in), (rel_sb, rel_in),
                             (dinv_sb, dinv_in), (w1a_sb, w1a_in),
                             (w1wbd_sb, w1wbd_in), (pqrM_sb, pqrM_in),
                             (iota8_sb, iota8_in)):
                nc.sync.dma_start(out=dst[:], in_=src[:])
            make_identity(nc, ident_sb[:])

            accum = bigp.tile([128, NB * 48], F32)
            R1 = bigp.tile([128, NB * 48], F32)
            out01 = bigp.tile([128, NB * 48], F32)
            saccum = bigp.tile([128, NB], F32)
            ns_sb = bigp.tile([128, NB], F32)
            pqr_sb = bigp.tile([128, NB * 3], F32)

            # phase A: T1R1; table <- dinv*T1; keep R1
            for b in range(NB):
                xbt = gpool.tile([FIN + 1, 128], F32, tag="xbt")
                nc.sync.dma_start(out=xbt[:], in_=xbT_in[:, b * 128:(b + 1) * 128])
                ps = psA.tile([128, 2 * KH], F32, tag="mm")
                nc.tensor.matmul(out=ps[:], lhsT=xbt[:], rhs=w1a_sb[:],
                                 start=True, stop=True)
                ev = gpool.tile([128, 48], BF16, tag="ev")
                nc.vector.tensor_scalar_mul(out=ev[:], in0=ps[:, 0:KH],
                                            scalar1=dinv_sb[:, b:b + 1])
                nc.sync.dma_start(out=tshard[b * CNT:(b + 1) * CNT, 0:KH],
                                  in_=ev[0:CNT, :])
                nc.vector.tensor_copy(out=R1[:, b * 48:(b + 1) * 48],
                                      in_=ps[:, KH:2 * KH])

            def allgather():
                nc.gpsimd.collective_compute(
                    "AllGather", OP.bypass, replica_groups=[list(range(NC))],
                    ins=[tshard[:]], outs=[tbl[:]])

            def post_prop(dst):
                a3 = accum[:].rearrange("p (b f) -> p b f", f=48)
                d3 = (dinv_sb[:].rearrange("p (b o) -> p b o", o=1)
                      .to_broadcast([128, NB, 48]))
                nc.vector.tensor_tensor(out=a3, in0=a3, in1=d3, op=OP.mult)
                nc.vector.tensor_tensor(out=dst[:], in0=accum[:], in1=R1[:],
                                        op=OP.add)
                nc.vector.tensor_scalar_max(out=dst[:], in0=dst[:], scalar1=0.0)

            allgather()
            _spmm_round(nc, psA, gpool, tbl, idx_sb, rel_sb, iota8_sb, accum, 48)
            post_prop(out01)

            # T2 = out0 @ w1wbd -> table
            for b in range(NB):
                pst = psB.tile([KH, 128], F32, tag="tr")
                nc.tensor.transpose(out=pst[:], in_=out01[:, b * 48:(b + 1) * 48],
                                    identity=ident_sb[:])
                sbt = gpool.tile([KH, 128], F32, tag="sbt")
                nc.vector.tensor_copy(out=sbt[:], in_=pst[:])
                ps2 = psB.tile([128, KH], F32, tag="mm2")
                nc.tensor.matmul(out=ps2[:], lhsT=sbt[:], rhs=w1wbd_sb[:],
                                 start=True, stop=True)
                ev = gpool.tile([128, 48], BF16, tag="ev")
                nc.vector.tensor_scalar_mul(out=ev[:], in0=ps2[:],
                                            scalar1=dinv_sb[:, b:b + 1])
                nc.sync.dma_start(out=tshard[b * CNT:(b + 1) * CNT, 0:KH],
                                  in_=ev[0:CNT, :])

            allgather()
            _spmm_round(nc, psA, gpool, tbl, idx_sb, rel_sb, iota8_sb, accum, 48)
            post_prop(out01)

            # pqr = out1 @ pqrM
            for b in range(NB):
                pst = psB.tile([KH, 128], F32, tag="tr")
                nc.tensor.transpose(out=pst[:], in_=out01[:, b * 48:(b + 1) * 48],
                                    identity=ident_sb[:])
                sbt = gpool.tile([KH, 128], F32, tag="sbt")
                nc.vector.tensor_copy(out=sbt[:], in_=pst[:])
                ps3 = psB.tile([128, 3], F32, tag="mm3")
                nc.tensor.matmul(out=ps3[:], lhsT=sbt[:], rhs=pqrM_sb[:],
                                 start=True, stop=True)
                nc.vector.tensor_copy(out=pqr_sb[:, b * 3:(b + 1) * 3],
                                      in_=ps3[:])

            pqr3 = pqr_sb[:].rearrange("p (b f) -> p b f", f=3)

            def write_scalar_table(src_tile):
                # hi/lo bf16 split for ~f32 accuracy through bf16 table
                hi = gpool.tile([128, NB], BF16, tag="hi")
                hif = gpool.tile([128, NB], F32, tag="hif")
                lo = gpool.tile([128, NB], BF16, tag="lo")
                hl = gpool.tile([128, NB * 2], BF16, tag="hl")
                nc.vector.tensor_copy(out=hi[:], in_=src_tile[:])
                nc.vector.tensor_copy(out=hif[:], in_=hi[:])
                nc.vector.tensor_tensor(out=hif[:], in0=src_tile[:], in1=hif[:],
                                        op=OP.subtract)
                nc.vector.tensor_copy(out=lo[:], in_=hif[:])
                hl3 = hl[:].rearrange("p (b l) -> p b l", l=2)
                nc.vector.tensor_copy(
                    out=hl3[:, :, 0:1],
                    in_=hi[:].rearrange("p (b o) -> p b o", o=1))
                nc.vector.tensor_copy(
                    out=hl3[:, :, 1:2],
                    in_=lo[:].rearrange("p (b o) -> p b o", o=1))
                nc.sync.dma_start(
                    out=tshard[:].rearrange("(b r) d -> r b d", r=CNT)[:, :, 0:2],
                    in_=hl[0:CNT, :].rearrange("r (b l) -> r b l", l=2))

            # spmv 1: table col0 = dinv * p
            pval = gpool.tile([128, NB], F32, tag="pv")
            nc.vector.tensor_tensor(out=pval[:], in0=pqr3[:, :, 0],
                                    in1=dinv_sb[:], op=OP.mult)
            write_scalar_table(pval)
            allgather()
            _spmm_round(nc, psA, gpool, tbl, idx_sb, rel_sb, iota8_sb, saccum, 2)

            # y = dinv*(dinv*s1 + q + dbar)
            yv = gpool.tile([128, NB], F32, tag="pv")
            nc.vector.tensor_tensor(out=yv[:], in0=saccum[:], in1=dinv_sb[:],
                                    op=OP.mult)
            qd = gpool.tile([128, NB], F32, tag="qd")
            nc.vector.tensor_scalar(out=qd[:], in0=pqr3[:, :, 1], scalar1=dbar,
                                    scalar2=None, op0=OP.add)
            nc.vector.tensor_tensor(out=yv[:], in0=yv[:], in1=qd[:], op=OP.add)
            nc.vector.tensor_tensor(out=yv[:], in0=yv[:], in1=dinv_sb[:],
                                    op=OP.mult)
            write_scalar_table(yv)
            allgather()
            _spmm_round(nc, psA, gpool, tbl, idx_sb, rel_sb, iota8_sb, saccum, 2)

            # node_scalar = dinv*s2 + r + ebar
            nc.vector.tensor_tensor(out=ns_sb[:], in0=saccum[:], in1=dinv_sb[:],
                                    op=OP.mult)
            rv = gpool.tile([128, NB], F32, tag="qd")
            nc.vector.tensor_scalar(out=rv[:], in0=pqr3[:, :, 2], scalar1=ebar,
                                    scalar2=None, op0=OP.add)
            nc.vector.tensor_tensor(out=ns_sb[:], in0=ns_sb[:], in1=rv[:],
                                    op=OP.add)
            nc.sync.dma_start(out=out_ext[:], in_=ns_sb[:])

    nc.compile()
    return nc


def kernel(**inputs):
    x = np.asarray(inputs["x"], np.float32)
    edge_index = np.asarray(inputs["edge_index"])
    batch = np.asarray(inputs["batch"]).astype(np.int64)
    w = {kk: np.asarray(vv, np.float32) for kk, vv in inputs.items()
         if kk not in ("x", "edge_index", "batch")}
    cores, shared = _host_prep(x, edge_index, batch, w)

    if "nc" not in _graph_cache:
        _graph_cache["nc"] = _build_graph(shared["dbar"], shared["ebar"])
    nc = _graph_cache["nc"]

    import ml_dtypes
    iota8 = np.broadcast_to(
        np.tile(np.arange(128, dtype=ml_dtypes.bfloat16), 8)[None, :],
        (128, 8 * 128)).copy()
    in_maps = []
    for c in range(NC):
        d = cores[c]
        in_maps.append({
            "idx": d["idx"], "rel": d["rel"], "dinv": d["dinv"],
            "xbT": d["xbT"], "w1a": shared["w1a"], "w1wbd": shared["w1wbd"],
            "pqrM": shared["pqrM"], "iota8": iota8,
        })
    global LAST_EXEC_NS
    res = run_bass_kernel_spmd(nc, in_maps, core_ids=list(range(NC)),
                               trace=TRACE)
    LAST_EXEC_NS = res.exec_time_ns

    pooled = np.zeros(G, np.float64)
    for c in range(NC):
        ns = res.results[c]["out"]           # [128, NB]
        real = cores[c]["real"]
        nid = cores[c]["nid"]
        gids = batch[nid[real]]
        pooled += np.bincount(gids, weights=ns[real].astype(np.float64),
                              minlength=G)
    out = pooled.astype(np.float32)[:, None] + shared["bg"]
    return out.astype(np.float32)
